# revision 6
# baseline (speedup 1.0000x reference)
"""Multi-head attention layer (B=8, N=1024, E=1024, H=16, D=64) on 8 TRN2
NeuronCores, data-parallel over batch (one batch element per core, weights
replicated, no collectives).

V6: qkT matmuls are emitted AFTER each iteration's scores+AV so the tile
scheduler slots them into exp-wait bubbles (ScalarE exp is the pacer inside
scores regions); qkT gets a dedicated PSUM buffer so score-tile rotation
never waits on its release. Weights/x load via single multi-dim DMA
descriptors (descriptor generation at ~0.6us each was serializing startup).
All bias matmuls removed (V-bias folded into out-proj bias on host; qk bias
via DVE tensor_scalar_add; out bias via DVE tensor_add of a broadcast tile).
V4 = 279.6us, V5 = 268.6us.
"""

import os

import ml_dtypes
import numpy as np

B, N, E, H, D = 8, 1024, 1024, 16, 64
P = 128
KE = E // P
NT = N // P
MQK = 2 * E // P
DP1 = D + 1

TRACE = os.environ.get("BASS_KERNEL_TRACE", "0") == "1"
LAST_EXEC_NS = None
LAST_RESULT = None

_COMPILED = None


def _build():
    import concourse.bass as bass
    import concourse.tile as tile
    from concourse import bacc, mybir

    f32 = mybir.dt.float32
    bf16 = mybir.dt.bfloat16
    AF = mybir.ActivationFunctionType
    MS = bass.MemorySpace

    nc = bacc.Bacc(
        "TRN2", target_bir_lowering=False, debug=False, enable_asserts=True
    )

    xT_d = nc.dram_tensor("xT", [E, N], bf16, kind="ExternalInput")
    wqkT_d = nc.dram_tensor("wqkT", [E, 2 * E], bf16, kind="ExternalInput")
    qkb_d = nc.dram_tensor("qkb", [P, MQK], f32, kind="ExternalInput")
    wvT_d = nc.dram_tensor("wvT", [E, E], bf16, kind="ExternalInput")
    woT_d = nc.dram_tensor("woT", [E, E], bf16, kind="ExternalInput")
    ob_d = nc.dram_tensor("ob", [1, E], f32, kind="ExternalInput")
    y_d = nc.dram_tensor("y", [N, E], f32, kind="ExternalOutput")

    with tile.TileContext(nc) as tc, tc.tile_pool(name="persist", bufs=1) as persist:
        qkT = [
            persist.tile([P, N], bf16, tag=f"qkT{m}", name=f"qkT{m}")
            for m in range(MQK)
        ]
        v_sb = [
            persist.tile([P, H * DP1], bf16, tag=f"v{m}", name=f"v{m}")
            for m in range(NT)
        ]
        attnT = [
            [
                persist.tile([P, 512], bf16, tag=f"attnT{k}_{ih}", name=f"attnT{k}_{ih}")
                for ih in range(2)
            ]
            for k in range(KE)
        ]
        woa = persist.tile([P, KE * E], bf16, tag="woa", name="woa")
        qkb_sb = persist.tile([P, MQK], f32, tag="qkb", name="qkb_sb")
        bias_bc = persist.tile([P, E], f32, tag="bias_bc", name="bias_bc")

        with (
            tc.tile_pool(name="ldx", bufs=1) as ldx,
            tc.tile_pool(name="ldqk", bufs=1) as ldqk,
            tc.tile_pool(name="psum", bufs=2, space=MS.PSUM) as psum_pool,
            tc.tile_pool(name="psqk", bufs=1, space=MS.PSUM) as psqk_pool,
            tc.tile_pool(name="pat", bufs=2, space=MS.PSUM) as pat_pool,
        ):
            xa = ldx.tile([P, KE * N], bf16, tag="xa", name="xa")
            wqa = ldqk.tile([P, KE * 2 * E], bf16, tag="wqa", name="wqa")
            xa3 = xa[:].rearrange("p (k n) -> p k n", n=N)
            xs3 = xT_d[:, :].rearrange("(k p) n -> p k n", p=P)
            wq3 = wqa[:].rearrange("p (k n) -> p k n", n=2 * E)
            wqs3 = wqkT_d[:, :].rearrange("(k p) n -> p k n", p=P)
            wo3 = woa[:].rearrange("p (k n) -> p k n", n=E)
            wos3 = woT_d[:, :].rearrange("(k p) n -> p k n", p=P)

            # ---------------- stage 2 first: v (needs only xa + wvT) --------
            with tc.tile_pool(name="ldv", bufs=1) as ldv:
                wva = ldv.tile([P, KE * E], bf16, tag="wva", name="wva")
                wv3 = wva[:].rearrange("p (k n) -> p k n", n=E)
                wvs3 = wvT_d[:, :].rearrange("(k p) n -> p k n", p=P)

                # Few big multi-dim descriptors; first V m-tile needs only
                # the m=0 x columns + the first wvT half (1.25MB).
                nc.sync.dma_start(xa3[:, :, 0:P], xs3[:, :, 0:P])
                nc.sync.dma_start(wv3[:, :, 0:512], wvs3[:, :, 0:512])
                nc.sync.dma_start(wv3[:, :, 512:E], wvs3[:, :, 512:E])
                nc.sync.dma_start(xa3[:, :, P:512], xs3[:, :, P:512])
                nc.sync.dma_start(xa3[:, :, 512:N], xs3[:, :, 512:N])
                nc.scalar.dma_start(qkb_sb[:], qkb_d[:, :])
                nc.scalar.dma_start(wq3[:, :, :], wqs3[:, :, :])
                nc.gpsimd.dma_start(bias_bc[:], ob_d[0:1, :].to_broadcast((P, E)))
                nc.scalar.dma_start(wo3[:, :, :], wos3[:, :, :])

                for m in range(NT):
                    ps = psum_pool.tile([P, N], f32, tag="big", name="psv")
                    for nh in range(2):
                        nsl = slice(nh * 512, (nh + 1) * 512)
                        for k in range(KE):
                            nc.tensor.matmul(
                                ps[:, nsl],
                                xa[:, k * N + m * P : k * N + (m + 1) * P],
                                wva[:, k * E + nh * 512 : k * E + (nh + 1) * 512],
                                start=(k == 0),
                                stop=(k == KE - 1),
                            )
                    src3 = ps[:].rearrange("p (h c) -> p h c", c=D)
                    dst3 = v_sb[m][:].rearrange("p (h c) -> p h c", c=DP1)
                    nc.scalar.copy(dst3[:, :, 0:D], src3)
                    nc.vector.memset(dst3[:, :, D : D + 1], 1.0)

            # ---------------- interleaved qkT + attention -------------------
            with (
                tc.tile_pool(name="probs", bufs=16) as probs_pool,
                tc.tile_pool(name="staged", bufs=5) as staged_pool,
                tc.tile_pool(name="bcast", bufs=4) as bcast_pool,
                tc.tile_pool(name="sums", bufs=2) as sums_pool,
                tc.tile_pool(name="ysb", bufs=2) as y_pool,
                tc.tile_pool(name="dram", bufs=4, space=MS.DRAM) as dram_pool,
            ):

                def qkT_mtile(m):
                    ps = psqk_pool.tile([P, N], f32, tag="qk", name="psqk")
                    for nh in range(2):
                        nsl = slice(nh * 512, (nh + 1) * 512)
                        for k in range(KE):
                            nc.tensor.matmul(
                                ps[:, nsl],
                                wqa[:, k * 2 * E + m * P : k * 2 * E + (m + 1) * P],
                                xa[:, k * N + nh * 512 : k * N + (nh + 1) * 512],
                                start=(k == 0),
                                stop=(k == KE - 1),
                            )
                    nc.vector.tensor_scalar_add(
                        qkT[m][:], ps[:], qkb_sb[:, m : m + 1]
                    )

                def scores_block(hp, ih):
                    isl = slice(ih * 512, (ih + 1) * 512)
                    qt, kt = qkT[hp], qkT[8 + hp]
                    pts = []
                    for jt in range(NT):
                        ps = psum_pool.tile([P, N], f32, tag="big", name="psc")
                        jsl = slice(jt * P, (jt + 1) * P)
                        nc.tensor.matmul(
                            ps[:, 0:512], kt[0:64, jsl], qt[0:64, isl],
                            start=True, stop=True,
                        )
                        nc.tensor.matmul(
                            ps[:, 512:1024], kt[64:128, jsl], qt[64:128, isl],
                            start=True, stop=True,
                        )
                        pt = probs_pool.tile([P, N], bf16, tag="probs", name="probs")
                        nc.scalar.activation(pt[:], ps[:], AF.Exp)
                        pts.append(pt)
                    return pts

                def av_accum(hp, ih, pts):
                    """Matmuls + staging + reciprocal + broadcast issue.

                    The attnT muls are deferred to av_finish so the broadcast
                    DMA latency never head-of-line-blocks later DVE work.
                    """
                    stg = []
                    rd = dram_pool.tile(
                        [2, 512], f32, tag="recip_dram", name="recip_dram"
                    )
                    sums = sums_pool.tile([2, 512], f32, tag="sums", name="sums")
                    recip = sums_pool.tile([2, 512], f32, tag="recip", name="recip")
                    for hs in range(2):
                        h = 2 * hp + hs
                        pa = pat_pool.tile([DP1, 512], f32, tag="pat", name="pat")
                        for jt in range(NT):
                            nc.tensor.matmul(
                                pa[:],
                                v_sb[jt][:, h * DP1 : (h + 1) * DP1],
                                pts[jt][:, hs * 512 : (hs + 1) * 512],
                                start=(jt == 0),
                                stop=(jt == NT - 1),
                            )
                        st = staged_pool.tile(
                            [DP1, 512], f32, tag="staged", name="staged"
                        )
                        nc.vector.tensor_copy(st[:], pa[:])
                        nc.sync.dma_start(sums[hs : hs + 1, :], st[D : D + 1, :])
                        stg.append(st)
                    nc.vector.reciprocal_approx_fast(recip[:], sums[:])
                    nc.sync.dma_start(rd[:], recip[:])
                    bcs = []
                    for hs in range(2):
                        bc = bcast_pool.tile([D, 512], f32, tag="bcast", name="bcast")
                        nc.gpsimd.dma_start(
                            bc[:], rd[hs : hs + 1, :].to_broadcast((D, 512))
                        )
                        bcs.append(bc)
                    return (hp, ih, stg, bcs)

                def av_finish(fin):
                    hp, ih, stg, bcs = fin
                    for hs in range(2):
                        base = hs * 64
                        nc.vector.tensor_mul(
                            attnT[hp][ih][base : base + 64, :],
                            stg[hs][0:D, :],
                            bcs[hs][:],
                        )

                def out_proj(ih, cs=range(4)):
                    for c in cs:
                        mi = ih * 4 + c
                        ps = psum_pool.tile([P, E], f32, tag="big", name="psy")
                        for nh in range(2):
                            nsl = slice(nh * 512, (nh + 1) * 512)
                            for k in range(KE):
                                nc.tensor.matmul(
                                    ps[:, nsl],
                                    attnT[k][ih][:, c * P : (c + 1) * P],
                                    woa[:, k * E + nh * 512 : k * E + (nh + 1) * 512],
                                    start=(k == 0),
                                    stop=(k == KE - 1),
                                )
                        ysb = y_pool.tile([P, E], f32, tag="ysb", name="ysb")
                        nc.vector.tensor_add(ysb[:], ps[:], bias_bc[:])
                        nc.sync.dma_start(y_d[mi * P : (mi + 1) * P, :], ysb[:])

                qkT_mtile(0)
                qkT_mtile(8)
                # qkT tiles are emitted AFTER each iteration's scores+AV so
                # the scheduler uses their matmuls as filler during exp
                # waits; the dedicated psqk buffer keeps the score-psum
                # rotation independent.
                pend_acc = None  # (hp, ih, pts) awaiting AV matmuls
                pend_fin = None  # accum result awaiting attnT muls
                for hp in range(8):
                    pts0 = scores_block(hp, 0)
                    new_fin = av_accum(*pend_acc) if pend_acc else None
                    if pend_fin:
                        av_finish(pend_fin)  # (hp-1, 0)
                    pts1 = scores_block(hp, 1)
                    fin0 = av_accum(hp, 0, pts0)
                    if new_fin:
                        av_finish(new_fin)  # (hp-1, 1)
                    if hp < 7:
                        qkT_mtile(hp + 1)
                        qkT_mtile(8 + hp + 1)
                    pend_fin = fin0
                    pend_acc = (hp, 1, pts1)
                fin1 = av_accum(*pend_acc)  # (7, 1)
                av_finish(pend_fin)  # (7, 0) — muls overlap accum(7,1) matmuls
                out_proj(0)
                av_finish(fin1)  # (7, 1)
                out_proj(1)

    nc.compile()
    return nc


def _prep_inputs(x, qkv_w, qkv_b, out_w, out_b):
    bf = ml_dtypes.bfloat16
    scale = np.float32(D ** -0.5)

    wq = (qkv_w[:E] * scale).astype(np.float32)
    wk = qkv_w[E : 2 * E]
    wv = qkv_w[2 * E :]
    wqkT = np.concatenate([wq, wk], axis=0).T.astype(bf)
    wvT = np.ascontiguousarray(wv.T).astype(bf)
    woT = np.ascontiguousarray(out_w.T).astype(bf)

    qkb = np.concatenate([qkv_b[:E] * scale, qkv_b[E : 2 * E]]).astype(np.float32)
    qkb = np.ascontiguousarray(qkb.reshape(MQK, P).T)
    # V-bias passes through softmax unchanged (rows sum to 1), so fold it
    # into the out-proj bias: y = attn0 @ Wo^T + (b_o + Wo @ b_v).
    vb = qkv_b[2 * E :].astype(np.float64)
    ob = (out_b.astype(np.float64) + out_w.astype(np.float64) @ vb).astype(
        np.float32
    ).reshape(1, E)

    in_maps = []
    for b in range(B):
        xT = np.ascontiguousarray(x[b].T).astype(bf)
        in_maps.append(
            {
                "xT": xT,
                "wqkT": wqkT,
                "qkb": qkb,
                "wvT": wvT,
                "woT": woT,
                "ob": ob,
            }
        )
    return in_maps


def _ensure_ntff_hook():
    import sys
    import types

    try:
        from antenv.axon_hooks import get_axon_ntff_profile_hook  # noqa: F401

        return
    except ImportError:
        pass
    try:
        from trn_agent_boot.trn_boot import _ntff_profile_via_ctypes

        hook = _ntff_profile_via_ctypes("/opt/axon/libaxon_pjrt.so")
    except Exception:
        hook = None
    mod = types.ModuleType("antenv.axon_hooks")
    mod.get_axon_ntff_profile_hook = lambda: hook
    sys.modules["antenv.axon_hooks"] = mod


def kernel(x, qkv_w, qkv_b, out_w, out_b):
    global _COMPILED, LAST_EXEC_NS, LAST_RESULT
    from concourse.bass_utils import run_bass_kernel_spmd

    if TRACE:
        _ensure_ntff_hook()

    if _COMPILED is None:
        _COMPILED = _build()
    nc = _COMPILED

    in_maps = _prep_inputs(
        np.asarray(x, np.float32),
        np.asarray(qkv_w, np.float32),
        np.asarray(qkv_b, np.float32),
        np.asarray(out_w, np.float32),
        np.asarray(out_b, np.float32),
    )

    res = run_bass_kernel_spmd(nc, in_maps, core_ids=list(range(B)), trace=TRACE)
    LAST_RESULT = res
    LAST_EXEC_NS = res.exec_time_ns

    y = np.stack([np.asarray(res.results[c]["y"]) for c in range(B)], axis=0)
    return y.astype(np.float32)


# revision 10
# speedup vs baseline: 1.2298x; 1.2298x over previous
"""Multi-head attention layer (B=8, N=1024, E=1024, H=16, D=64) on 8 TRN2
NeuronCores, data-parallel over batch (one batch element per core, weights
replicated, no collectives).

V6: qkT matmuls are emitted AFTER each iteration's scores+AV so the tile
scheduler slots them into exp-wait bubbles (ScalarE exp is the pacer inside
scores regions); qkT gets a dedicated PSUM buffer so score-tile rotation
never waits on its release. Weights/x load via single multi-dim DMA
descriptors (descriptor generation at ~0.6us each was serializing startup).
All bias matmuls removed (V-bias folded into out-proj bias on host; qk bias
via DVE tensor_scalar_add; out bias via DVE tensor_add of a broadcast tile).
V4 = 279.6us, V5 = 268.6us.
"""

import os

import ml_dtypes
import numpy as np

B, N, E, H, D = 8, 1024, 1024, 16, 64
P = 128
KE = E // P
NT = N // P
MQK = 2 * E // P
DP1 = D + 1

TRACE = os.environ.get("BASS_KERNEL_TRACE", "0") == "1"
LAST_EXEC_NS = None
LAST_RESULT = None

_COMPILED = None


def _build():
    import concourse.bass as bass
    import concourse.tile as tile
    from concourse import bacc, mybir

    f32 = mybir.dt.float32
    bf16 = mybir.dt.bfloat16
    AF = mybir.ActivationFunctionType
    MS = bass.MemorySpace

    nc = bacc.Bacc(
        "TRN2", target_bir_lowering=False, debug=False, enable_asserts=True
    )

    xT_d = nc.dram_tensor("xT", [E, N], bf16, kind="ExternalInput")
    wqkT_d = nc.dram_tensor("wqkT", [E, 2 * E], bf16, kind="ExternalInput")
    qkb_d = nc.dram_tensor("qkb", [P, MQK], f32, kind="ExternalInput")
    wvT_d = nc.dram_tensor("wvT", [E, E], bf16, kind="ExternalInput")
    woT_d = nc.dram_tensor("woT", [E, E], bf16, kind="ExternalInput")
    ob_d = nc.dram_tensor("ob", [1, E], f32, kind="ExternalInput")
    y_d = nc.dram_tensor("y", [N, E], f32, kind="ExternalOutput")

    with tile.TileContext(nc) as tc, tc.tile_pool(name="persist", bufs=1) as persist:
        qkT = [
            persist.tile([P, N], bf16, tag=f"qkT{m}", name=f"qkT{m}")
            for m in range(MQK)
        ]
        v_sb = [
            persist.tile([P, H * DP1], bf16, tag=f"v{m}", name=f"v{m}")
            for m in range(NT)
        ]
        attnT = [
            [
                persist.tile([P, 512], bf16, tag=f"attnT{k}_{ih}", name=f"attnT{k}_{ih}")
                for ih in range(2)
            ]
            for k in range(KE)
        ]
        woa = persist.tile([P, KE * E], bf16, tag="woa", name="woa")
        qkb_sb = persist.tile([P, MQK], f32, tag="qkb", name="qkb_sb")
        bias_bc = persist.tile([P, E], f32, tag="bias_bc", name="bias_bc")

        with (
            tc.tile_pool(name="ldx", bufs=1) as ldx,
            tc.tile_pool(name="ldqk", bufs=1) as ldqk,
            tc.tile_pool(name="psum", bufs=3, space=MS.PSUM) as psum_pool,
            tc.tile_pool(name="pat", bufs=2, space=MS.PSUM) as pat_pool,
        ):
            xa = ldx.tile([P, KE * N], bf16, tag="xa", name="xa")
            wqa = ldqk.tile([P, KE * 2 * E], bf16, tag="wqa", name="wqa")
            xa3 = xa[:].rearrange("p (k n) -> p k n", n=N)
            xs3 = xT_d[:, :].rearrange("(k p) n -> p k n", p=P)
            wq3 = wqa[:].rearrange("p (k n) -> p k n", n=2 * E)
            wqs3 = wqkT_d[:, :].rearrange("(k p) n -> p k n", p=P)
            wo3 = woa[:].rearrange("p (k n) -> p k n", n=E)
            wos3 = woT_d[:, :].rearrange("(k p) n -> p k n", p=P)

            # ---------------- stage 2 first: v (needs only xa + wvT) --------
            with tc.tile_pool(name="ldv", bufs=1) as ldv:
                wva = ldv.tile([P, KE * E], bf16, tag="wva", name="wva")
                wv3 = wva[:].rearrange("p (k n) -> p k n", n=E)
                wvs3 = wvT_d[:, :].rearrange("(k p) n -> p k n", p=P)

                # Few big multi-dim descriptors; first V m-tile needs only
                # the m=0 x columns + the first wvT half (1.25MB).
                # All big loads serialized on the SP sequencer in priority
                # order so later transfers enter the DMA queues later — the
                # first V m-tile needs only the m=0 x columns + first wvT
                # half (1.25MB).
                nc.sync.dma_start(xa3[:, :, 0:P], xs3[:, :, 0:P])
                nc.sync.dma_start(wv3[:, :, 0:512], wvs3[:, :, 0:512])
                nc.sync.dma_start(wv3[:, :, 512:E], wvs3[:, :, 512:E])
                nc.sync.dma_start(xa3[:, :, P:512], xs3[:, :, P:512])
                nc.sync.dma_start(xa3[:, :, 512:N], xs3[:, :, 512:N])
                nc.sync.dma_start(wq3[:, :, 0:E], wqs3[:, :, 0:E])
                nc.sync.dma_start(wq3[:, :, E : 2 * E], wqs3[:, :, E : 2 * E])
                nc.sync.dma_start(wo3[:, :, :], wos3[:, :, :])
                nc.scalar.dma_start(qkb_sb[:], qkb_d[:, :])
                nc.gpsimd.dma_start(bias_bc[:], ob_d[0:1, :].to_broadcast((P, E)))

                for m in range(NT):
                    ps = psum_pool.tile([P, N], f32, tag="big", name="psv")
                    for nh in range(2):
                        nsl = slice(nh * 512, (nh + 1) * 512)
                        for k in range(KE):
                            nc.tensor.matmul(
                                ps[:, nsl],
                                xa[:, k * N + m * P : k * N + (m + 1) * P],
                                wva[:, k * E + nh * 512 : k * E + (nh + 1) * 512],
                                start=(k == 0),
                                stop=(k == KE - 1),
                            )
                    src3 = ps[:].rearrange("p (h c) -> p h c", c=D)
                    dst3 = v_sb[m][:].rearrange("p (h c) -> p h c", c=DP1)
                    nc.scalar.copy(dst3[:, :, 0:D], src3)
                    nc.vector.memset(dst3[:, :, D : D + 1], 1.0)

            # ---------------- interleaved qkT + attention -------------------
            with (
                tc.tile_pool(name="probs", bufs=16) as probs_pool,
                tc.tile_pool(name="staged", bufs=5) as staged_pool,
                tc.tile_pool(name="bcast", bufs=4) as bcast_pool,
                tc.tile_pool(name="sums", bufs=2) as sums_pool,
                tc.tile_pool(name="ysb", bufs=2) as y_pool,
                tc.tile_pool(name="dram", bufs=4, space=MS.DRAM) as dram_pool,
            ):

                def qkT_half(m, nh, ps):
                    nsl = slice(nh * 512, (nh + 1) * 512)
                    for k in range(KE):
                        nc.tensor.matmul(
                            ps[:, nsl],
                            wqa[:, k * 2 * E + m * P : k * 2 * E + (m + 1) * P],
                            xa[:, k * N + nh * 512 : k * N + (nh + 1) * 512],
                            start=(k == 0),
                            stop=(k == KE - 1),
                        )
                    if nh == 1:
                        nc.vector.tensor_scalar_add(
                            qkT[m][:], ps[:], qkb_sb[:, m : m + 1]
                        )

                def qkT_mtile(m):
                    ps = psum_pool.tile([P, N], f32, tag="big", name="psqk")
                    qkT_half(m, 0, ps)
                    qkT_half(m, 1, ps)

                def scores_block(hp, ih):
                    isl = slice(ih * 512, (ih + 1) * 512)
                    qt, kt = qkT[hp], qkT[8 + hp]
                    pts = []
                    for jt in range(NT):
                        ps = psum_pool.tile([P, N], f32, tag="big", name="psc")
                        jsl = slice(jt * P, (jt + 1) * P)
                        nc.tensor.matmul(
                            ps[:, 0:512], kt[0:64, jsl], qt[0:64, isl],
                            start=True, stop=True,
                        )
                        nc.tensor.matmul(
                            ps[:, 512:1024], kt[64:128, jsl], qt[64:128, isl],
                            start=True, stop=True,
                        )
                        pt = probs_pool.tile([P, N], bf16, tag="probs", name="probs")
                        nc.scalar.activation(pt[:], ps[:], AF.Exp)
                        pts.append(pt)
                    return pts

                def av_accum(hp, ih, pts):
                    """Matmuls + staging + reciprocal + broadcast issue.

                    The attnT muls are deferred to av_finish so the broadcast
                    DMA latency never head-of-line-blocks later DVE work.
                    """
                    stg = []
                    rd = dram_pool.tile(
                        [2, 512], f32, tag="recip_dram", name="recip_dram"
                    )
                    sums = sums_pool.tile([2, 512], f32, tag="sums", name="sums")
                    recip = sums_pool.tile([2, 512], f32, tag="recip", name="recip")
                    for hs in range(2):
                        h = 2 * hp + hs
                        pa = pat_pool.tile([DP1, 512], f32, tag="pat", name="pat")
                        for jt in range(NT):
                            nc.tensor.matmul(
                                pa[:],
                                v_sb[jt][:, h * DP1 : (h + 1) * DP1],
                                pts[jt][:, hs * 512 : (hs + 1) * 512],
                                start=(jt == 0),
                                stop=(jt == NT - 1),
                            )
                        st = staged_pool.tile(
                            [DP1, 512], f32, tag="staged", name="staged"
                        )
                        nc.vector.tensor_copy(st[:], pa[:])
                        nc.sync.dma_start(sums[hs : hs + 1, :], st[D : D + 1, :])
                        stg.append(st)
                    nc.vector.reciprocal_approx_fast(recip[:], sums[:])
                    nc.sync.dma_start(rd[:], recip[:])
                    bcs = []
                    for hs in range(2):
                        bc = bcast_pool.tile([D, 512], f32, tag="bcast", name="bcast")
                        nc.gpsimd.dma_start(
                            bc[:], rd[hs : hs + 1, :].to_broadcast((D, 512))
                        )
                        bcs.append(bc)
                    return (hp, ih, stg, bcs)

                def av_finish(fin):
                    hp, ih, stg, bcs = fin
                    for hs in range(2):
                        base = hs * 64
                        nc.vector.tensor_mul(
                            attnT[hp][ih][base : base + 64, :],
                            stg[hs][0:D, :],
                            bcs[hs][:],
                        )

                def out_proj(ih, cs=range(4)):
                    for c in cs:
                        mi = ih * 4 + c
                        ps = psum_pool.tile([P, E], f32, tag="big", name="psy")
                        for nh in range(2):
                            nsl = slice(nh * 512, (nh + 1) * 512)
                            for k in range(KE):
                                nc.tensor.matmul(
                                    ps[:, nsl],
                                    attnT[k][ih][:, c * P : (c + 1) * P],
                                    woa[:, k * E + nh * 512 : k * E + (nh + 1) * 512],
                                    start=(k == 0),
                                    stop=(k == KE - 1),
                                )
                        ysb = y_pool.tile([P, E], f32, tag="ysb", name="ysb")
                        nc.vector.tensor_add(ysb[:], ps[:], bias_bc[:])
                        nc.sync.dma_start(y_d[mi * P : (mi + 1) * P, :], ysb[:])

                qkT_mtile(0)
                qkT_mtile(8)
                # qkT m-tiles are emitted in nh-halves at 4 points per
                # iteration so ScalarE's exp stream is never starved by a
                # long score-free matmul block.
                pend_acc = None  # (hp, ih, pts) awaiting AV matmuls
                pend_fin = None  # accum result awaiting attnT muls
                for hp in range(8):
                    psq0 = psq1 = None
                    pts0 = scores_block(hp, 0)
                    if hp < 7:
                        psq0 = psum_pool.tile([P, N], f32, tag="big", name="psqk")
                        qkT_half(hp + 1, 0, psq0)
                    new_fin = av_accum(*pend_acc) if pend_acc else None
                    if psq0 is not None:
                        qkT_half(hp + 1, 1, psq0)
                    if pend_fin:
                        av_finish(pend_fin)  # (hp-1, 0)
                    pts1 = scores_block(hp, 1)
                    if hp < 7:
                        psq1 = psum_pool.tile([P, N], f32, tag="big", name="psqk")
                        qkT_half(8 + hp + 1, 0, psq1)
                    fin0 = av_accum(hp, 0, pts0)
                    if psq1 is not None:
                        qkT_half(8 + hp + 1, 1, psq1)
                    if new_fin:
                        av_finish(new_fin)  # (hp-1, 1)
                    pend_fin = fin0
                    pend_acc = (hp, 1, pts1)
                fin1 = av_accum(*pend_acc)  # (7, 1)
                av_finish(pend_fin)  # (7, 0) — muls overlap accum(7,1) matmuls
                out_proj(0)
                av_finish(fin1)  # (7, 1)
                out_proj(1)

    nc.compile()
    return nc


def _prep_inputs(x, qkv_w, qkv_b, out_w, out_b):
    bf = ml_dtypes.bfloat16
    scale = np.float32(D ** -0.5)

    wq = (qkv_w[:E] * scale).astype(np.float32)
    wk = qkv_w[E : 2 * E]
    wv = qkv_w[2 * E :]
    wqkT = np.concatenate([wq, wk], axis=0).T.astype(bf)
    wvT = np.ascontiguousarray(wv.T).astype(bf)
    woT = np.ascontiguousarray(out_w.T).astype(bf)

    qkb = np.concatenate([qkv_b[:E] * scale, qkv_b[E : 2 * E]]).astype(np.float32)
    qkb = np.ascontiguousarray(qkb.reshape(MQK, P).T)
    # V-bias passes through softmax unchanged (rows sum to 1), so fold it
    # into the out-proj bias: y = attn0 @ Wo^T + (b_o + Wo @ b_v).
    vb = qkv_b[2 * E :].astype(np.float64)
    ob = (out_b.astype(np.float64) + out_w.astype(np.float64) @ vb).astype(
        np.float32
    ).reshape(1, E)

    in_maps = []
    for b in range(B):
        xT = np.ascontiguousarray(x[b].T).astype(bf)
        in_maps.append(
            {
                "xT": xT,
                "wqkT": wqkT,
                "qkb": qkb,
                "wvT": wvT,
                "woT": woT,
                "ob": ob,
            }
        )
    return in_maps


def _ensure_ntff_hook():
    import sys
    import types

    try:
        from antenv.axon_hooks import get_axon_ntff_profile_hook  # noqa: F401

        return
    except ImportError:
        pass
    try:
        from trn_agent_boot.trn_boot import _ntff_profile_via_ctypes

        hook = _ntff_profile_via_ctypes("/opt/axon/libaxon_pjrt.so")
    except Exception:
        hook = None
    mod = types.ModuleType("antenv.axon_hooks")
    mod.get_axon_ntff_profile_hook = lambda: hook
    sys.modules["antenv.axon_hooks"] = mod


def kernel(x, qkv_w, qkv_b, out_w, out_b):
    global _COMPILED, LAST_EXEC_NS, LAST_RESULT
    from concourse.bass_utils import run_bass_kernel_spmd

    if TRACE:
        _ensure_ntff_hook()

    if _COMPILED is None:
        _COMPILED = _build()
    nc = _COMPILED

    in_maps = _prep_inputs(
        np.asarray(x, np.float32),
        np.asarray(qkv_w, np.float32),
        np.asarray(qkv_b, np.float32),
        np.asarray(out_w, np.float32),
        np.asarray(out_b, np.float32),
    )

    res = run_bass_kernel_spmd(nc, in_maps, core_ids=list(range(B)), trace=TRACE)
    LAST_RESULT = res
    LAST_EXEC_NS = res.exec_time_ns

    y = np.stack([np.asarray(res.results[c]["y"]) for c in range(B)], axis=0)
    return y.astype(np.float32)
